# revision 10
# baseline (speedup 1.0000x reference)
"""CosFace loss kernel for Trainium2 (Bass/Tile), 8-core data-parallel.

Reference computation (per full input logits [B, C] f32, labels [B] int):
    t[b]   = logits[b, labels[b]]                    (target logit gather)
    u[b]   = t[b] - M
    sin_theta[b]        = sin(acos(t[b]))  == sqrt((1-t)(1+t))
    sin_theta_plus_m[b] = sin(acos(u[b]))  == sqrt((1-u)(1+u))
    sin_m[b]            = sin(acos(u)-acos(t)) == sin_tpm*t - u*sin_theta
    diff[b, j] = S * (logits[b, j + (j >= labels[b])] - u[b]),  j in [0, C-1)

Sharding: batch-dim across 8 cores, R = B/8 = 64 rows per core. On-chip
tiles are [128, W]: partition p = (row r, column-half k) so that all 128
SBUF partitions are used; the column-drop shift j -> j+1 stays inside the
free dim because each partition holds a contiguous slice of one row.

Per core, per chunk c0 (W columns of each column-half):
    load  x_t[128, W+1]  (overlapping +1 load gives both x[j] and x[j+1])
    mask  = (iota_f >= lab_adj - c0)          (gpsimd tensor_scalar)
    y     = x_t[:, 0:W]; y[mask] = x_t[:, 1:W+1]   (DVE copy + copy_predicated)
    yf    = S*y + (S*M - S*t)                 (ACT activation, per-part. bias)
    store yf
The row tail j = C-1 computes garbage and is dropped on the host; the input
is padded by PAD floats so the final shifted read stays in bounds.

The target gather t is one indirect (embedding-style) DMA using on-device
computed flat offsets  offs[p] = (p>>1)*C + labels[p>>1].
"""

import os
import sys

import numpy as np

for _p in ("/opt/trn_rl_repo", "/root/.axon_site/_ro/trn_rl_repo"):
    if os.path.isdir(_p) and _p not in sys.path:
        sys.path.insert(0, _p)

import concourse.bass as bass
import concourse.bacc as bacc
import concourse.mybir as mybir
import concourse.tile as tile
from concourse.bass_utils import run_bass_kernel_spmd

S = 64.0
M = 0.35
B, C = 512, 100000
NCORES = 8
R = B // NCORES  # rows per core
PAD = 128
W_DEFAULT = 2500

f32 = mybir.dt.float32
i32 = mybir.dt.int32
u8 = mybir.dt.uint8
Alu = mybir.AluOpType
Act = mybir.ActivationFunctionType


def build_nc(r=R, c=C, w=W_DEFAULT, pad=PAD, n_devices=NCORES):
    """Build the per-core Bass program (identical on every core)."""
    assert c % 2 == 0
    half = c // 2
    assert half % w == 0
    nch = half // w
    p = 2 * r  # partitions in use

    nc = bacc.Bacc("TRN2", target_bir_lowering=False, debug=False,
                   num_devices=n_devices)
    x = nc.dram_tensor("x", [r * c + pad], f32, kind="ExternalInput")
    lab = nc.dram_tensor("lab", [p, 1], i32, kind="ExternalInput")
    out = nc.dram_tensor("out", [r * c], f32, kind="ExternalOutput")
    small = nc.dram_tensor("small", [p, 3], f32, kind="ExternalOutput")

    sm_bias = float(S * M)

    with tile.TileContext(nc) as tc:
        with tc.tile_pool(name="consts", bufs=1) as consts, \
             tc.tile_pool(name="io", bufs=3) as iopool, \
             tc.tile_pool(name="work", bufs=3) as wpool:
            # ---- setup: index vectors, label thresholds, target gather ----
            iota_f = consts.tile([p, w], f32)
            nc.gpsimd.iota(iota_f[:], pattern=[[1, w]], base=0,
                           channel_multiplier=0,
                           allow_small_or_imprecise_dtypes=True)
            iota_p = consts.tile([p, 1], i32)
            nc.gpsimd.iota(iota_p[:], pattern=[[0, 1]], base=0,
                           channel_multiplier=1)
            lab_t = consts.tile([p, 1], i32)
            nc.sync.dma_start(lab_t[:], lab[:])

            # kbase[p] = (p & 1) * half ; lab_adj = label - kbase
            kpar = consts.tile([p, 1], i32)
            nc.vector.tensor_scalar(kpar[:], iota_p[:], 1, None,
                                    Alu.bitwise_and)
            kbase = consts.tile([p, 1], i32)
            nc.gpsimd.tensor_scalar(kbase[:], kpar[:], half, None, Alu.mult)
            lab_adj_i = consts.tile([p, 1], i32)
            nc.gpsimd.tensor_sub(lab_adj_i[:], lab_t[:], kbase[:])
            lab_adj = consts.tile([p, 1], f32)
            nc.gpsimd.tensor_copy(lab_adj[:], lab_adj_i[:])

            # offs[p] = (p >> 1) * c + label[p]  (flat element index into x)
            rhalf = consts.tile([p, 1], i32)
            nc.vector.tensor_scalar(rhalf[:], iota_p[:], 1, None,
                                    Alu.arith_shift_right)
            rowbase = consts.tile([p, 1], i32)
            nc.gpsimd.tensor_scalar(rowbase[:], rhalf[:], c, None, Alu.mult)
            offs = consts.tile([p, 1], i32)
            nc.gpsimd.tensor_add(offs[:], rowbase[:], lab_t[:])

            tg = consts.tile([p, 1], f32)  # t, duplicated per partition pair
            n_tot = r * c + pad
            nc.gpsimd.indirect_dma_start(
                out=tg[:],
                out_offset=None,
                in_=bass.AP(x, 0, [[n_tot, 1], [1, n_tot]]),
                in_offset=bass.IndirectOffsetOnAxis(ap=offs[:], axis=1),
            )

            # bias[p] = S*M - S*t
            bias_t = consts.tile([p, 1], f32)
            nc.gpsimd.tensor_scalar(bias_t[:], tg[:], -S, sm_bias,
                                    Alu.mult, Alu.add)

            # ---- small outputs: sin_theta, sin_theta_plus_m, sin_m ----
            small_t = consts.tile([p, 3], f32)
            u_t = consts.tile([p, 1], f32)
            nc.gpsimd.tensor_scalar(u_t[:], tg[:], -M, None, Alu.add)
            tmp0 = consts.tile([p, 1], f32)
            tmp1 = consts.tile([p, 1], f32)
            prod = consts.tile([p, 1], f32)
            # sin_theta = sqrt((1-t)(1+t))
            nc.gpsimd.tensor_scalar(tmp0[:], tg[:], -1.0, 1.0,
                                    Alu.mult, Alu.add)
            nc.gpsimd.tensor_scalar(tmp1[:], tg[:], 1.0, None, Alu.add)
            nc.gpsimd.tensor_mul(prod[:], tmp0[:], tmp1[:])
            nc.scalar.sqrt(small_t[:, 0:1], prod[:])
            # sin_theta_plus_m = sqrt((1-u)(1+u))
            tmp2 = consts.tile([p, 1], f32)
            tmp3 = consts.tile([p, 1], f32)
            prod2 = consts.tile([p, 1], f32)
            nc.gpsimd.tensor_scalar(tmp2[:], u_t[:], -1.0, 1.0,
                                    Alu.mult, Alu.add)
            nc.gpsimd.tensor_scalar(tmp3[:], u_t[:], 1.0, None, Alu.add)
            nc.gpsimd.tensor_mul(prod2[:], tmp2[:], tmp3[:])
            nc.scalar.sqrt(small_t[:, 1:2], prod2[:])
            # sin_m = sin_tpm * t - u * sin_theta
            pa = consts.tile([p, 1], f32)
            pb = consts.tile([p, 1], f32)
            nc.gpsimd.tensor_mul(pa[:], small_t[:, 1:2], tg[:])
            nc.gpsimd.tensor_mul(pb[:], u_t[:], small_t[:, 0:1])
            nc.gpsimd.tensor_sub(small_t[:, 2:3], pa[:], pb[:])
            nc.sync.dma_start(small[:], small_t[:])

            # ---- main streaming pass over column chunks ----
            for ci in range(nch):
                c0 = ci * w
                x_t = iopool.tile([p, w + 1], f32, tag="x")
                nc.sync.dma_start(
                    x_t[:], bass.AP(x, c0, [[c, r], [half, 2], [1, w + 1]]))

                thr = wpool.tile([p, 1], f32, tag="thr")
                nc.gpsimd.tensor_scalar(thr[:], lab_adj[:], float(c0), None,
                                        Alu.subtract)
                mask = wpool.tile([p, w], u8, tag="mask")
                nc.gpsimd.tensor_scalar(mask[:], iota_f[:], thr[:], None,
                                        Alu.is_ge)

                y = wpool.tile([p, w], f32, tag="y")
                nc.vector.tensor_copy(y[:], x_t[:, 0:w])
                nc.vector.copy_predicated(y[:], mask[:], x_t[:, 1:w + 1])

                yf = wpool.tile([p, w], f32, tag="yf")
                nc.scalar.activation(yf[:], y[:], Act.Identity,
                                     bias=bias_t[:], scale=S)
                nc.sync.dma_start(
                    bass.AP(out, c0, [[c, r], [half, 2], [1, w]]), yf[:])
    nc.compile()
    return nc


_NC_CACHE = {}


def _get_nc():
    key = (R, C, W_DEFAULT)
    if key not in _NC_CACHE:
        _NC_CACHE[key] = build_nc()
    return _NC_CACHE[key]


def shard_inputs(logits, labels):
    logits = np.ascontiguousarray(np.asarray(logits, dtype=np.float32))
    labels = np.asarray(labels).astype(np.int32)
    in_maps = []
    for i in range(NCORES):
        x = np.zeros(R * C + PAD, np.float32)
        x[:R * C] = logits[i * R:(i + 1) * R].reshape(-1)
        lab = np.repeat(labels[i * R:(i + 1) * R], 2).reshape(2 * R, 1)
        in_maps.append({"x": x, "lab": np.ascontiguousarray(lab, np.int32)})
    return in_maps


def assemble_outputs(results):
    diff = np.empty((B, C - 1), np.float32)
    smalls = np.empty((B, 3), np.float32)
    for i, res in enumerate(results):
        diff[i * R:(i + 1) * R] = res["out"].reshape(R, C)[:, :C - 1]
        smalls[i * R:(i + 1) * R] = res["small"].reshape(2 * R, 3)[0::2]
    return (diff, smalls[:, 0].copy(), smalls[:, 1].copy(),
            smalls[:, 2].copy())


def kernel(logits, labels, **run_kwargs):
    nc = _get_nc()
    in_maps = shard_inputs(logits, labels)
    br = run_bass_kernel_spmd(nc, in_maps, core_ids=list(range(NCORES)),
                              **run_kwargs)
    out = assemble_outputs(br.results)
    kernel.last_results = br
    return out


# revision 11
# speedup vs baseline: 67373.9985x; 67373.9985x over previous
"""CosFace loss kernel for Trainium2 (Bass/Tile), 8-core data-parallel.

Reference computation (per full input logits [B, C] f32, labels [B] int):
    t[b]   = logits[b, labels[b]]                    (target logit gather)
    u[b]   = t[b] - M
    sin_theta[b]        = sin(acos(t[b]))  == sqrt((1-t)(1+t))
    sin_theta_plus_m[b] = sin(acos(u[b]))  == sqrt((1-u)(1+u))
    sin_m[b]            = sin(acos(u)-acos(t)) == sin_tpm*t - u*sin_theta
    diff[b, j] = S * (logits[b, j + (j >= labels[b])] - u[b]),  j in [0, C-1)

Sharding: batch-dim across 8 cores, R = B/8 = 64 rows per core. On-chip
tiles are [128, W]: partition p = (row r, column-half k) so that all 128
SBUF partitions are used; the column-drop shift j -> j+1 stays inside the
free dim because each partition holds a contiguous slice of one row.

Per core, per chunk c0 (W columns of each column-half):
    load  x_t[128, W+1]  (overlapping +1 load gives both x[j] and x[j+1])
    mask  = (iota_f >= lab_adj - c0)          (gpsimd tensor_scalar)
    y     = x_t[:, 0:W]; y[mask] = x_t[:, 1:W+1]   (DVE copy + copy_predicated)
    yf    = S*y + (S*M - S*t)                 (ACT activation, per-part. bias)
    store yf
The row tail j = C-1 computes garbage and is dropped on the host; the input
is padded by PAD floats so the final shifted read stays in bounds.

The target gather t is one indirect (embedding-style) DMA using on-device
computed flat offsets  offs[p] = (p>>1)*C + labels[p>>1].
"""

import os
import sys

import numpy as np

for _p in ("/opt/trn_rl_repo", "/root/.axon_site/_ro/trn_rl_repo"):
    if os.path.isdir(_p) and _p not in sys.path:
        sys.path.insert(0, _p)

import concourse.bass as bass
import concourse.bacc as bacc
import concourse.mybir as mybir
import concourse.tile as tile
from concourse.bass_utils import run_bass_kernel_spmd

S = 64.0
M = 0.35
B, C = 512, 100000
NCORES = 8
R = B // NCORES  # rows per core
PAD = 128
W_DEFAULT = 2500

f32 = mybir.dt.float32
i32 = mybir.dt.int32
u8 = mybir.dt.uint8
Alu = mybir.AluOpType
Act = mybir.ActivationFunctionType


def build_nc(r=R, c=C, w=W_DEFAULT, pad=PAD, n_devices=NCORES,
             io_bufs=3, w_bufs=3):
    """Build the per-core Bass program (identical on every core)."""
    assert c % 2 == 0
    half = c // 2
    assert half % w == 0
    nch = half // w
    p = 2 * r  # partitions in use

    nc = bacc.Bacc("TRN2", target_bir_lowering=False, debug=False,
                   num_devices=n_devices)
    x = nc.dram_tensor("x", [r * c + pad], f32, kind="ExternalInput")
    lab = nc.dram_tensor("lab", [p, 1], i32, kind="ExternalInput")
    out = nc.dram_tensor("out", [r * c], f32, kind="ExternalOutput")
    small = nc.dram_tensor("small", [p, 3], f32, kind="ExternalOutput")

    sm_bias = float(S * M)

    with tile.TileContext(nc) as tc:
        with tc.tile_pool(name="consts", bufs=1) as consts, \
             tc.tile_pool(name="io", bufs=io_bufs) as iopool, \
             tc.tile_pool(name="work", bufs=w_bufs) as wpool:
            # ---- setup: index vectors, label thresholds, target gather ----
            iota_f = consts.tile([p, w], f32)
            nc.gpsimd.iota(iota_f[:], pattern=[[1, w]], base=0,
                           channel_multiplier=0,
                           allow_small_or_imprecise_dtypes=True)
            iota_p = consts.tile([p, 1], i32)
            nc.gpsimd.iota(iota_p[:], pattern=[[0, 1]], base=0,
                           channel_multiplier=1)
            lab_t = consts.tile([p, 1], i32)
            nc.sync.dma_start(lab_t[:], lab[:])

            # kbase[p] = (p & 1) * half ; lab_adj = label - kbase
            kpar = consts.tile([p, 1], i32)
            nc.vector.tensor_scalar(kpar[:], iota_p[:], 1, None,
                                    Alu.bitwise_and)
            kbase = consts.tile([p, 1], i32)
            nc.gpsimd.tensor_scalar(kbase[:], kpar[:], half, None, Alu.mult)
            lab_adj_i = consts.tile([p, 1], i32)
            nc.gpsimd.tensor_sub(lab_adj_i[:], lab_t[:], kbase[:])
            lab_adj = consts.tile([p, 1], f32)
            nc.gpsimd.tensor_copy(lab_adj[:], lab_adj_i[:])

            # offs[p] = (p >> 1) * c + label[p]  (flat element index into x)
            rhalf = consts.tile([p, 1], i32)
            nc.vector.tensor_scalar(rhalf[:], iota_p[:], 1, None,
                                    Alu.arith_shift_right)
            rowbase = consts.tile([p, 1], i32)
            nc.gpsimd.tensor_scalar(rowbase[:], rhalf[:], c, None, Alu.mult)
            offs = consts.tile([p, 1], i32)
            nc.gpsimd.tensor_add(offs[:], rowbase[:], lab_t[:])

            tg = consts.tile([p, 1], f32)  # t, duplicated per partition pair
            n_tot = r * c + pad
            nc.gpsimd.indirect_dma_start(
                out=tg[:],
                out_offset=None,
                in_=bass.AP(x, 0, [[n_tot, 1], [1, n_tot]]),
                in_offset=bass.IndirectOffsetOnAxis(ap=offs[:], axis=1),
            )

            # bias[p] = S*M - S*t
            bias_t = consts.tile([p, 1], f32)
            nc.gpsimd.tensor_scalar(bias_t[:], tg[:], -S, sm_bias,
                                    Alu.mult, Alu.add)

            # ---- small outputs: sin_theta, sin_theta_plus_m, sin_m ----
            small_t = consts.tile([p, 3], f32)
            u_t = consts.tile([p, 1], f32)
            nc.gpsimd.tensor_scalar(u_t[:], tg[:], -M, None, Alu.add)
            tmp0 = consts.tile([p, 1], f32)
            tmp1 = consts.tile([p, 1], f32)
            prod = consts.tile([p, 1], f32)
            # sin_theta = sqrt((1-t)(1+t))
            nc.gpsimd.tensor_scalar(tmp0[:], tg[:], -1.0, 1.0,
                                    Alu.mult, Alu.add)
            nc.gpsimd.tensor_scalar(tmp1[:], tg[:], 1.0, None, Alu.add)
            nc.gpsimd.tensor_mul(prod[:], tmp0[:], tmp1[:])
            nc.scalar.sqrt(small_t[:, 0:1], prod[:])
            # sin_theta_plus_m = sqrt((1-u)(1+u))
            tmp2 = consts.tile([p, 1], f32)
            tmp3 = consts.tile([p, 1], f32)
            prod2 = consts.tile([p, 1], f32)
            nc.gpsimd.tensor_scalar(tmp2[:], u_t[:], -1.0, 1.0,
                                    Alu.mult, Alu.add)
            nc.gpsimd.tensor_scalar(tmp3[:], u_t[:], 1.0, None, Alu.add)
            nc.gpsimd.tensor_mul(prod2[:], tmp2[:], tmp3[:])
            nc.scalar.sqrt(small_t[:, 1:2], prod2[:])
            # sin_m = sin_tpm * t - u * sin_theta
            pa = consts.tile([p, 1], f32)
            pb = consts.tile([p, 1], f32)
            nc.gpsimd.tensor_mul(pa[:], small_t[:, 1:2], tg[:])
            nc.gpsimd.tensor_mul(pb[:], u_t[:], small_t[:, 0:1])
            nc.gpsimd.tensor_sub(small_t[:, 2:3], pa[:], pb[:])
            nc.sync.dma_start(small[:], small_t[:])

            # ---- main streaming pass over column chunks ----
            for ci in range(nch):
                c0 = ci * w
                x_t = iopool.tile([p, w + 1], f32, tag="x")
                nc.sync.dma_start(
                    x_t[:], bass.AP(x, c0, [[c, r], [half, 2], [1, w + 1]]))

                thr = wpool.tile([p, 1], f32, tag="thr")
                nc.gpsimd.tensor_scalar(thr[:], lab_adj[:], float(c0), None,
                                        Alu.subtract)
                mask = wpool.tile([p, w], u8, tag="mask")
                nc.gpsimd.tensor_scalar(mask[:], iota_f[:], thr[:], None,
                                        Alu.is_ge)

                y = wpool.tile([p, w], f32, tag="y")
                nc.vector.tensor_copy(y[:], x_t[:, 0:w])
                nc.vector.copy_predicated(y[:], mask[:], x_t[:, 1:w + 1])

                yf = wpool.tile([p, w], f32, tag="yf")
                nc.scalar.activation(yf[:], y[:], Act.Identity,
                                     bias=bias_t[:], scale=S)
                nc.sync.dma_start(
                    bass.AP(out, c0, [[c, r], [half, 2], [1, w]]), yf[:])
    nc.compile()
    return nc


_NC_CACHE = {}


def _get_nc():
    key = (R, C, W_DEFAULT)
    if key not in _NC_CACHE:
        _NC_CACHE[key] = build_nc()
    return _NC_CACHE[key]


def shard_inputs(logits, labels):
    logits = np.ascontiguousarray(np.asarray(logits, dtype=np.float32))
    labels = np.asarray(labels).astype(np.int32)
    in_maps = []
    for i in range(NCORES):
        x = np.zeros(R * C + PAD, np.float32)
        x[:R * C] = logits[i * R:(i + 1) * R].reshape(-1)
        lab = np.repeat(labels[i * R:(i + 1) * R], 2).reshape(2 * R, 1)
        in_maps.append({"x": x, "lab": np.ascontiguousarray(lab, np.int32)})
    return in_maps


def assemble_outputs(results):
    diff = np.empty((B, C - 1), np.float32)
    smalls = np.empty((B, 3), np.float32)
    for i, res in enumerate(results):
        diff[i * R:(i + 1) * R] = res["out"].reshape(R, C)[:, :C - 1]
        smalls[i * R:(i + 1) * R] = res["small"].reshape(2 * R, 3)[0::2]
    return (diff, smalls[:, 0].copy(), smalls[:, 1].copy(),
            smalls[:, 2].copy())


def kernel(logits, labels, **run_kwargs):
    nc = _get_nc()
    in_maps = shard_inputs(logits, labels)
    br = run_bass_kernel_spmd(nc, in_maps, core_ids=list(range(NCORES)),
                              **run_kwargs)
    out = assemble_outputs(br.results)
    kernel.last_results = br
    return out
